# revision 33
# baseline (speedup 1.0000x reference)
"""Trainium2 Bass kernel: causal multi-head attention (B=2, N=2048, DIM=1024, H=16, DH=64).

Sharding over 8 NeuronCores: data-parallel on batch (2) x tensor-parallel on
head groups (4 heads / core).  Each core computes Q/K/V projections for its 4
heads, causal flash-style attention, and a partial output projection against
its slice of Wo.  The 4 partial outputs per batch are summed (plus bo) on the
host to form the full output.

Layout notes (per core):
  - x arrives pre-transposed and pre-cast from the host as xt = bf16(x[b].T)
    (DIM, N) so the contraction dim of every projection matmul sits on SBUF
    partitions and the load is half the bytes.
  - Q^T / K^T are kept with head-dim on partitions: pair tensors (128, 2, N)
    where partitions 0:64 hold head 2p and 64:128 hold head 2p+1.  The two
    heads of a pair issue row-tiled (tile_position) matmuls on the PE array.
  - Scores are computed transposed: S^T (k_seq on partitions, q on free), so
    softmax needs no max subtraction (scores ~ N(0,1)) and P^T feeds the
    P@V matmul directly with K=128.  Row sums l come for free from a ones
    column appended to V (lhsT = [V | 1], out rows 0:64 = O^T, row 64 = l).
  - A warmup burst of zero matmuls runs during the input DMA so the PE HAM
    clock-gate reaches K=8/8 before the real projections issue, and dummy
    matmuls keep it warm through ACT-bound stretches near the tail.
  - The input DMA is split in two waves (all transfers race for bandwidth,
    so wave 2 gets a WAW dependency on a sliver memset to hold it back
    until wave 1 lands).
  - All PE work besides attention (projections for later chunks, lagged
    output projections) is queued as accumulation-chain items popped one
    per attention k-tile, so the in-order PE queue interleaves attention
    with fill work instead of idling during ACT-bound stretches or
    starving the scalar engine during projection blocks.
  - Normalization bounces reciprocals through DRAM (partition-broadcast
    DMA) where attention still runs behind it; the exposed last pair uses
    a short chain instead: l spread to 8 partitions by DMA, one 8-lane
    reciprocal, and bf16 selector matmuls broadcasting r on the PE.
"""

import numpy as np
import ml_dtypes

import concourse.bass as bass
import concourse.bacc as bacc
import concourse.tile as tile
from concourse import mybir
from concourse.bass_utils import run_bass_kernel_spmd

B, N, DIM, H, DH = 2, 2048, 1024, 16, 64
HG = 4                  # heads per core
GROUPS = 4              # tensor-parallel degree (head groups)
GCOLS = HG * DH         # 256 inner columns per core
NKT = DIM // 128        # 8 contraction tiles for projections
NQC = N // 512          # 4 query chunks
NMT = N // 128          # 16 sequence tiles
SCALE = DH ** -0.5

# Schraudolph fast-exp constants (DVE): bits(exp(s*SCALE)) ~= A*s + B as
# int32; the bf16 hi-half of the int32 is the result.  The magic offset only
# shifts the mean error, which softmax normalization cancels; the residual
# is ~1.8% RMS on p, applied to half the heads.
EXP_A = SCALE * 1.4426950408889634 * (1 << 23)
EXP_B = (127.0 - 0.043677448) * (1 << 23)

F32 = mybir.dt.float32
BF16 = mybir.dt.bfloat16
I32 = mybir.dt.int32


def build():
    nc = bacc.Bacc("TRN2", target_bir_lowering=False, debug=True)

    xt = nc.declare_dram_parameter("xt", [DIM, N], BF16, isOutput=False)
    wq = nc.declare_dram_parameter("wq", [DIM, GCOLS], BF16, isOutput=False)
    wk = nc.declare_dram_parameter("wk", [DIM, GCOLS], BF16, isOutput=False)
    wv = nc.declare_dram_parameter("wv", [DIM, GCOLS], BF16, isOutput=False)
    wo = nc.declare_dram_parameter("wo", [GCOLS, DIM], BF16, isOutput=False)
    out = nc.declare_dram_parameter("out", [N, DIM], BF16, isOutput=True)

    with tile.TileContext(nc) as tc:
        with (
            tc.tile_pool(name="const", bufs=1) as const,
            tc.tile_pool(name="ptp", bufs=3) as ptp,
            tc.tile_pool(name="lp", bufs=2) as lp,
            tc.tile_pool(name="outs", bufs=6) as outs,
            tc.tile_pool(name="psS", bufs=2, space="PSUM") as psS,
            tc.tile_pool(name="psO", bufs=1, space="PSUM") as psO,
            tc.tile_pool(name="psP", bufs=2, space="PSUM") as psP,
            tc.tile_pool(name="dramp", bufs=1, space="DRAM") as dramp,
        ):
            # ---------------- persistent tiles ----------------
            wz = const.tile([128, 528], BF16)             # warmup zeros
            xT = const.tile([128, NKT, N], BF16)           # x^T (dim on partitions)
            wqsb = const.tile([128, NKT, GCOLS], BF16)
            wksb = const.tile([128, NKT, GCOLS], BF16)
            wvsb = const.tile([128, NKT, GCOLS], BF16)
            wosb = const.tile([128, 2, DIM], BF16)        # Wo rows, head-pair layout
            tmask = const.tile([128, 2, 128], BF16)       # triangular binary mask
            v1 = const.tile([128, NMT, HG, DH + 1], BF16)  # [V | ones]
            qth = const.tile([128, 2, N], BF16)           # Q^T head pairs
            kth = const.tile([128, 2, N], BF16)           # K^T head pairs
            ost = const.tile([65, HG, N], F32)            # unnormalized O^T + l row
            ones1 = const.tile([1, 64], BF16)             # K=1 PE-broadcast lhsT
            osb = const.tile([128, 2, N], BF16)           # normalized O^T, pair layout

            # -------- PE warmup + staged wave-1 loads ----------
            # All transfers queued together race concurrently across the DMA
            # engines, so completion tracks the TOTAL queued bytes.  Wave 1a
            # carries only the K projection's inputs (wk + x's first half,
            # 2.5MB); wq and wv are released mid-warmup — a tiny DVE copy of
            # a warmup PSUM tile paces the DVE queue to ~the right moment,
            # and a sliver memset over each weight tile gives its DMA a WAW
            # dependency — so each lands just before its projections need it
            # instead of all racing to finish together.
            nc.vector.memset(wz[:, :], 0.0)
            xsrc = xt[:, :].rearrange("(t p) n -> p t n", p=128)
            nc.sync.dma_start(
                out=wksb[:, :, :],
                in_=wk[:, :].rearrange("(t p) n -> p t n", p=128))
            nc.sync.dma_start(out=xT[:, :, 0:512], in_=xsrc[:, :, 0:512])

            def warm_mm():
                # small dummy matmul that keeps the PE HAM clock-gate at
                # K=8/8 through ACT-bound stretches with no real fill work
                psw = psP.tile([16, 512], F32, tag="proj", name="ps_warm")
                nc.tensor.matmul(psw[:, :], wz[:, 0:16], wz[:, 16:528],
                                 start=True, stop=True)
                return psw

            def warm_mm_cheap():
                # ultra-short dummy (F=32): keeps HAM activity without
                # stealing meaningful PE streaming time
                psw = psP.tile([16, 32], F32, tag="proj", name="ps_warmc")
                nc.tensor.matmul(psw[:, :], wz[:, 0:16], wz[:, 16:48],
                                 start=True, stop=True)
                return psw

            wpace = const.tile([16, 512], BF16)           # warmup pacing dst
            for w in range(14):
                psw = warm_mm()
                if w in (4, 6, 8):
                    nc.vector.tensor_copy(out=wpace[:, :], in_=psw[:, :])
                    if w == 4:
                        nc.vector.memset(wqsb[:, 0:1, 0:2], 0.0)
                        nc.sync.dma_start(
                            out=wqsb[:, :, :],
                            in_=wq[:, :].rearrange("(t p) n -> p t n", p=128))
                    elif w == 6:
                        nc.vector.memset(wvsb[:, 0:1, 0:2], 0.0)
                        nc.sync.dma_start(
                            out=wvsb[:, :, :],
                            in_=wv[:, :].rearrange("(t p) n -> p t n", p=128))
                    else:
                        nc.vector.memset(xT[:, 0:1, 512:514], 0.0)
                        nc.sync.dma_start(out=xT[:, :, 512:1024],
                                          in_=xsrc[:, :, 512:1024])

            def wave2():
                # tiny memsets over slivers of the wave-2 destinations give
                # the DMA triggers a WAW dependency on the DVE queue, so the
                # transfers only start once chunk 0's projection copies have
                # drained -- i.e. after wave 1 has finished, instead of
                # racing it for bandwidth
                nc.vector.memset(xT[:, 0:1, 1024:1026], 0.0)
                nc.vector.memset(wosb[:, :, 0:2], 0.0)
                for h in range(HG):
                    p, e = divmod(h, 2)
                    nc.sync.dma_start(out=wosb[e * 64:(e + 1) * 64, p, :],
                                      in_=wo[h * DH:(h + 1) * DH, :])
                nc.sync.dma_start(out=xT[:, :, 1024:2048],
                                  in_=xsrc[:, :, 1024:2048])

            # ones column of [V | 1]
            nc.vector.memset(v1[:, :, :, DH:DH + 1], 1.0)
            # triangular binary mask for the 128-wide diagonal boundary
            # sub-block (identical for every diagonal block): keep q >= k,
            # i.e. f - p >= 0
            nc.gpsimd.memset(tmask[:, :, :], 1.0)
            nc.gpsimd.affine_select(
                out=tmask[:, :, :], in_=tmask[:, :, :],
                compare_op=mybir.AluOpType.is_ge, fill=0.0,
                base=0, pattern=[[0, 2], [1, 128]],
                channel_multiplier=-1,
            )
            # ones row: ones1.T @ r broadcasts r's single partition onto 64
            # output partitions on the PE (K=1 matmul)
            nc.gpsimd.memset(ones1[:, :], 1.0)

            # ---------- phases 2-4 fused: proj + attention per q-chunk ------
            # PE work besides attention (projections for later chunks, lagged
            # output projections) is queued as whole-accumulation-chain items
            # and popped one per attention k-tile, so the in-order PE queue
            # alternates attention and fill work instead of serializing a
            # 10us projection block (during which the scalar engine starves)
            # or idling ~150ns per ACT-bound k-tile.
            pending = []

            def tick(dummy_fill=False):
                if pending:
                    pending.pop(0)()
                elif dummy_fill:
                    warm_mm_cheap()

            def flush():
                while pending:
                    pending.pop(0)()

            def qk_item(dst, wsb, pair, c):
                def emit():
                    cs = slice(c * 512, (c + 1) * 512)
                    pcols = slice(pair * 128, (pair + 1) * 128)
                    ps = psP.tile([128, 512], F32, tag="proj", name="ps_proj")
                    for k in range(NKT):
                        nc.tensor.matmul(ps[:, :], wsb[:, k, pcols], xT[:, k, cs],
                                         start=(k == 0), stop=(k == NKT - 1))
                    nc.vector.tensor_copy(out=dst[:, pair, cs], in_=ps[:, :])
                return emit

            def v_item(mt):
                def emit():
                    ms = slice(mt * 128, (mt + 1) * 128)
                    ps = psP.tile([128, GCOLS], F32, tag="proj", name="ps_v")
                    for k in range(NKT):
                        nc.tensor.matmul(ps[:, :], xT[:, k, ms], wvsb[:, k, :],
                                         start=(k == 0), stop=(k == NKT - 1))
                    nc.vector.tensor_copy(
                        out=v1[:, mt, :, 0:DH],
                        in_=ps[:, :].rearrange("p (h d) -> p h d", h=HG),
                    )
                return emit

            def proj_items(c):
                # K first (attention's first dependency), then Q, then V
                items = []
                for dst, wsb in ((kth, wksb), (qth, wqsb)):
                    for pair in range(2):
                        items.append(qk_item(dst, wsb, pair, c))
                for mt in range(4 * c, 4 * c + 4):
                    items.append(v_item(mt))
                return items

            def proj_chunk0_start():
                # emit only pair 0's critical path inline (K p0, Q p0,
                # V mt0) and queue the rest — attention pair 0 then starts
                # ~8us earlier, its ticks popping V mt1-3 and K p1 just in
                # time for their consumers
                it = proj_items(0)
                it[0]()
                for _ in range(3):
                    warm_mm()
                it[2](); it[4]()
                pending.extend([it[5], it[6], it[7], it[1], it[3]])

            rdram = dramp.tile([64, 128], F32)

            def oproj_item(mt, tail, evac="vector"):
                # one output-projection tile (partial): bo is added on the
                # host, so PSUM evacuation is a plain cast.  In the exposed
                # tail the evacuations split across the scalar engine (idle
                # once the last exp is done) and DVE, PSUM tiles alternate
                # between the psP ring and the now-idle psS ring (deeper
                # effective rotation), and the output DMA goes per-half so
                # it starts as soon as its half is evacuated.  evac="scalar"
                # keeps the whole evacuation off the DVE so PE ring-slot
                # releases don't queue behind a stalled DVE chain.
                def emit():
                    ms = slice(mt * 128, (mt + 1) * 128)
                    ot = outs.tile([128, 1024], BF16, tag="ot", name="ot")
                    for nh in range(2):
                        ns = slice(nh * 512, (nh + 1) * 512)
                        if tail and nh == 1:
                            ps = psS.tile([128, 512], F32, tag="s", name="ps_out")
                        else:
                            ps = psP.tile([128, 512], F32, tag="proj", name="ps_out")
                        for p in range(2):
                            nc.tensor.matmul(ps[:, :], osb[:, p, ms], wosb[:, p, ns],
                                             start=(p == 0), stop=(p == 1))
                        dst = ot[:, nh * 512:(nh + 1) * 512]
                        if tail:
                            nc.scalar.copy(out=dst[:, 0:256], in_=ps[:, 0:256])
                            nc.vector.tensor_copy(out=dst[:, 256:512],
                                                  in_=ps[:, 256:512])
                            nc.sync.dma_start(out=out[ms, ns],
                                              in_=ot[:, nh * 512:(nh + 1) * 512])
                        elif evac == "scalar":
                            nc.scalar.copy(out=dst, in_=ps[:, :])
                        else:
                            nc.vector.tensor_copy(out=dst, in_=ps[:, :])
                    if not tail:
                        nc.sync.dma_start(out=out[ms, :], in_=ot[:, :])
                return emit

            def out_proj(c, tail=False):
                for mt in range(4 * c, 4 * c + 4):
                    oproj_item(mt, tail)()

            def out_proj_tail(c, rcb):
                # While the last pair's reciprocal chain drains on the DVE,
                # the PE runs real outproj work: the two held-back full
                # items, then the first four groups' p=0 matmuls (pair 0
                # normalized long ago).  Exactly four groups stay open --
                # one per distinct bank slot across the psP and psS rings --
                # with their closers emitted after the normalize, so no ring
                # slot is reallocated while its group is open.
                oproj_item(4 * (c - 1) + 2, False, evac="scalar")()
                oproj_item(4 * (c - 1) + 3, False, evac="scalar")()
                mts = list(range(4 * c, 4 * c + 4))
                groups = [(mt, nh) for mt in mts[:2] for nh in range(2)]
                tiles = {}
                ots = {mt: outs.tile([128, 1024], BF16, tag="ot", name="ot")
                       for mt in mts[:2]}
                for gi, (mt, nh) in enumerate(groups):
                    ms = slice(mt * 128, (mt + 1) * 128)
                    ns = slice(nh * 512, (nh + 1) * 512)
                    pool = psP if gi % 2 == 0 else psS
                    tag = "proj" if gi % 2 == 0 else "s"
                    ps = pool.tile([128, 512], F32, tag=tag, name="ps_out")
                    tiles[(mt, nh)] = ps
                    nc.tensor.matmul(ps[:, :], osb[:, 0, ms], wosb[:, 0, ns],
                                     start=True, stop=False)
                # pad the in-order PE queue through the reciprocal chain's
                # DMA latency so the bcast matmuls don't expose it, and the
                # HAM clock-gate stays warm for the closers
                for _ in range(20):
                    warm_mm_cheap()
                tail_bcast_and_norm(c, 1, rcb)
                for mt, nh in groups:
                    ms = slice(mt * 128, (mt + 1) * 128)
                    ns = slice(nh * 512, (nh + 1) * 512)
                    ps = tiles[(mt, nh)]
                    nc.tensor.matmul(ps[:, :], osb[:, 1, ms], wosb[:, 1, ns],
                                     start=False, stop=True)
                    dst = ots[mt][:, nh * 512:(nh + 1) * 512]
                    nc.scalar.copy(out=dst[:, 0:256], in_=ps[:, 0:256])
                    nc.vector.tensor_copy(out=dst[:, 256:512],
                                          in_=ps[:, 256:512])
                    nc.sync.dma_start(out=out[ms, ns], in_=dst)
                for mt in mts[2:]:
                    oproj_item(mt, True)()

            def normalize_pair(c, pair):
                # softmax denominators for this (chunk, pair), then normalize
                # O^T.  The reciprocal rows bounce through DRAM
                # (partition-broadcast DMA, zero PE cost); the latency is
                # hidden by the attention still running behind this pair.
                qs = slice(c * 512, (c + 1) * 512)
                hA = 2 * pair
                lc = lp.tile([8, 128], F32, tag="lc", name="lc")
                for i, h in enumerate((hA, hA + 1)):
                    nc.sync.dma_start(out=lc[4 * i: 4 * i + 4, :],
                                      in_=ost[64:65, h, qs])
                rc32 = lp.tile([8, 128], F32, tag="rc32", name="rc32")
                nc.vector.reciprocal(out=rc32[:, :], in_=lc[:, :])
                rslot = rdram[c * 16 + 8 * pair: c * 16 + 8 * pair + 8, :]
                nc.sync.dma_start(out=rslot, in_=rc32[:, :])
                rb = lp.tile([64, 8, 128], F32, tag="rb", name="rb")
                bcast = bass.AP(tensor=rslot.tensor, offset=rslot.offset,
                                ap=[[0, 64]] + list(rslot.ap))
                nc.sync.dma_start(out=rb[:, :, :], in_=bcast)
                for i, h in enumerate((hA, hA + 1)):
                    r_src = rb[:, 4 * i: 4 * i + 4, :]
                    o_src = ost[0:64, h, qs].rearrange("p (s f) -> p s f", f=128)
                    if i == 0:
                        nc.vector.tensor_mul(
                            osb[0:64, pair, qs].rearrange("p (s f) -> p s f", f=128),
                            o_src, r_src)
                    else:
                        # odd head lives on partitions 64:128 of the pair
                        # tensor; DVE can't cross partitions, so stage + DMA
                        onst = outs.tile([64, 512], BF16, tag="onst", name="onst")
                        nc.vector.tensor_mul(
                            onst[:, :].rearrange("p (s f) -> p s f", f=128),
                            o_src, r_src)
                        nc.sync.dma_start(out=osb[64:128, pair, qs], in_=onst[:, :])

            def normalize_fast_chain(c, pair, poA, poB):
                # Exposed-tail variant: the l rows hop straight from PSUM to
                # a transposed [128, 2, 4] layout (DMA scatter), so the
                # iterative reciprocal runs 128 lanes wide (8 elems/lane)
                # instead of 1, then hop back to a [1, 2, 512] row for the
                # PE K=1 broadcast.  Returns the rcb tile; the broadcast +
                # multiply run later so real outproj work can fill the PE
                # while this chain drains.
                # Round-trip through a [128, 4] lane-spread: the element
                # order within the spread is irrelevant (elementwise recip),
                # only forward/back consistency matters, so the DMA AP
                # balancer's natural contiguous split is fine.
                qs = slice(c * 512, (c + 1) * 512)
                hA = 2 * pair
                lrt = lp.tile([32, 2, 16], F32, tag="lrt", name="lrt")
                for i, h in ((0, hA), (1, hA + 1)):
                    nc.sync.dma_start(out=lrt[:, i, :], in_=ost[64:65, h, qs])
                rT = lp.tile([32, 2, 16], BF16, tag="rT", name="rT")
                with nc.allow_low_precision(reason="r is rounded to bf16 for "
                                            "the PE broadcast rhs anyway"):
                    nc.vector.reciprocal(out=rT[:, :, :], in_=lrt[:, :, :])
                rcb = lp.tile([1, 2, 512], BF16, tag="rcb", name="rcbf")
                for i in range(2):
                    nc.sync.dma_start(out=rcb[:, i, :], in_=rT[:, i, :])
                return rcb

            def tail_bcast_and_norm(c, pair, rcb):
                # rb[64, 512] = ones1.T @ rcb row (K=1 matmul broadcast);
                # odd head first so its staging DMA overlaps the even
                # head's multiply.
                qs = slice(c * 512, (c + 1) * 512)
                hA = 2 * pair
                for i, h in ((1, hA + 1), (0, hA)):
                    # psO ring: free after the last pair's evacuation, and
                    # crucially NOT the psP ring the open p0 groups hold
                    rbp = psO.tile([64, 512], F32, tag="oB" if i else "oA",
                                   name="rb_ps")
                    nc.tensor.matmul(rbp[:, :], ones1[0:1, :], rcb[:, i, :],
                                     start=True, stop=True)
                    o_src = ost[0:64, h, qs]
                    if i == 0:
                        nc.vector.tensor_mul(osb[0:64, pair, qs], o_src,
                                             rbp[:, :])
                    else:
                        onst = outs.tile([64, 512], BF16, tag="onst", name="onst")
                        nc.vector.tensor_mul(onst[:, :], o_src, rbp[:, :])
                        nc.sync.dma_start(out=osb[64:128, pair, qs],
                                          in_=onst[:, :])

            def attn_pair(c, pair, fast=False, dummy_fill=False):
                qs = slice(c * 512, (c + 1) * 512)
                nkt = 4 * (c + 1)
                hA, hB = 2 * pair, 2 * pair + 1
                poA = psO.tile([65, 512], F32, tag="oA", name="poA")
                poB = psO.tile([65, 512], F32, tag="oB", name="poB")
                for k in range(nkt):
                    ks = slice(k * 128, (k + 1) * 128)
                    j = k - 4 * c
                    # on diagonal blocks, q columns f < 128*j are fully
                    # masked: skip their S^T stream, exp, and P@V
                    # accumulation entirely
                    fs = 128 * max(j, 0)
                    qsj = slice(c * 512 + fs, (c + 1) * 512)
                    ss = psS.tile([128, 1024], F32, tag="s", name="ss")
                    # S^T = K^T.T @ Q^T for both heads of the pair
                    # (row-tiled: head A rows 0:64, head B rows 64:128)
                    nc.tensor.matmul(ss[:, fs:512], kth[0:64, pair, ks],
                                     qth[0:64, pair, qsj],
                                     start=True, stop=True)
                    nc.tensor.matmul(ss[:, 512 + fs:1024], kth[64:128, pair, ks],
                                     qth[64:128, pair, qsj],
                                     start=True, stop=True)
                    pt = ptp.tile([128, 2, 512], BF16, tag="pt", name="pt")
                    if j < 0:
                        # below the diagonal: everything unmasked
                        nc.scalar.activation(out=pt[:, :, :], in_=ss[:, :],
                                             func=mybir.ActivationFunctionType.Exp,
                                             scale=SCALE)
                    else:
                        # diagonal block: exp the live columns, then zero
                        # the triangular boundary sub-block's upper part
                        nc.scalar.activation(
                            out=pt[:, :, fs:], in_=ss[:, :].rearrange(
                                "p (e f) -> p e f", e=2)[:, :, fs:],
                            func=mybir.ActivationFunctionType.Exp,
                            scale=SCALE)
                        nc.vector.tensor_mul(pt[:, :, fs:fs + 128],
                                             pt[:, :, fs:fs + 128],
                                             tmask[:, :, :])
                    nc.tensor.matmul(poA[:, fs:], v1[:, k, hA, :],
                                     pt[:, 0, fs:],
                                     start=(k == 0), stop=(k == nkt - 1))
                    nc.tensor.matmul(poB[:, fs:], v1[:, k, hB, :],
                                     pt[:, 1, fs:],
                                     start=(k == 0), stop=(k == nkt - 1))
                    tick(dummy_fill)
                if fast:
                    # exposed tail: evacuate the two heads on different
                    # engines so the copies run in parallel, then start the
                    # reciprocal chain; the caller fills the PE while it
                    # drains and finishes via tail_bcast_and_norm
                    nc.scalar.copy(out=ost[:, hB, qs], in_=poB[:, :])
                    nc.vector.tensor_copy(out=ost[:, hA, qs], in_=poA[:, :])
                    return normalize_fast_chain(c, pair, poA, poB)
                else:
                    nc.vector.tensor_copy(out=ost[:, hA, qs], in_=poA[:, :])
                    nc.vector.tensor_copy(out=ost[:, hB, qs], in_=poB[:, :])

            proj_chunk0_start()
            wave2()
            for c in range(NQC):
                last = c == NQC - 1
                if c > 0:
                    # this chunk's attention reads its own Q/K/V: finish any
                    # of its projection items still pending
                    flush()
                if not last:
                    # queue the next chunk's projections; they pop one per
                    # attention k-tile
                    if c == 0:
                        # chunk 0's own leftovers must land first: its pair-0
                        # ticks pop V mt1-3 + K p1, and Q p1 is flushed
                        # before pair 1 needs it
                        attn_pair(0, 0)
                        flush()
                        pending.extend(proj_items(1))
                    else:
                        pending.extend(proj_items(c + 1))
                if c == 0:
                    # chunk 0's pair 1 is all diagonal blocks (needs DVE
                    # mask-muls immediately), so normalize after both pairs
                    # to avoid a DVE head-of-line stall on the DRAM bounce
                    attn_pair(0, 1)
                    normalize_pair(0, 0)
                    normalize_pair(0, 1)
                else:
                    # out-proj items for all completed chunks are deferred to
                    # the last chunk's attention, the only ACT-bound stretch
                    # with spare PE slots (chunks 1-2 are already PE-bound
                    # with projection fill work).  Chunks 0-1 go in at pair-0
                    # start, chunk 2 at pair-1 start (so its normalize chain
                    # has drained by the time the items pop).
                    if last:
                        pending.extend(oproj_item(mt, False)
                                       for mt in range(0, 8))
                    attn_pair(c, 0, dummy_fill=last)
                    normalize_pair(c, 0)
                    if last:
                        pending.extend(oproj_item(mt, False)
                                       for mt in range(8, 10))
                    rcb = attn_pair(c, 1, fast=last, dummy_fill=last)
                    if not last:
                        normalize_pair(c, 1)
                    else:
                        flush()
            out_proj_tail(NQC - 1, rcb)

    nc.compile()
    return nc


def _in_maps(inputs):
    bf = ml_dtypes.bfloat16
    x = np.asarray(inputs["x"], np.float32)
    Wq = np.asarray(inputs["Wq"], np.float32).astype(bf)
    Wkv = np.asarray(inputs["Wkv"], np.float32).astype(bf)
    Wo = np.asarray(inputs["Wo"], np.float32).astype(bf)
    maps = []
    for i in range(8):
        b, g = divmod(i, GROUPS)
        cs = slice(g * GCOLS, (g + 1) * GCOLS)
        maps.append(dict(
            xt=np.ascontiguousarray(x[b].T.astype(bf)),
            wq=np.ascontiguousarray(Wq[:, cs]),
            wk=np.ascontiguousarray(Wkv[:, cs]),
            wv=np.ascontiguousarray(Wkv[:, DIM + g * GCOLS: DIM + (g + 1) * GCOLS]),
            wo=np.ascontiguousarray(Wo[cs, :]),
        ))
    return maps


_NC = None


def _get_nc():
    global _NC
    if _NC is None:
        nc = build()
        nc.finalize()
        _NC = nc
    return _NC


def run(inputs, trace=False, **kwargs):
    maps = _in_maps(inputs)
    res = run_bass_kernel_spmd(_get_nc(), maps, core_ids=list(range(8)),
                               trace=trace, **kwargs)
    bo = np.asarray(inputs["bo"], np.float32)
    out = np.empty((B, N, DIM), np.float32)
    for b in range(B):
        acc = res.results[4 * b]["out"].astype(np.float32)
        for g in range(1, GROUPS):
            acc = acc + res.results[4 * b + g]["out"].astype(np.float32)
        out[b] = acc + bo
    return out, res


def kernel(**inputs):
    out, _ = run(inputs, trace=False)
    return out

